# revision 1
# baseline (speedup 1.0000x reference)
"""Trainium2 Bass kernel v2 for nn_AttentionBlock (GroupNorm + MHA + proj + residual).

x: [16, 512, 32, 32] fp32, 8 cores data-parallel (2 images/core).

Key differences vs v1:
  * fp8(e4m3) + DoubleRow matmuls for PV, softmax-rowsum and proj (and
    optionally the QKV convs with KMODE=fp8): 2x contraction per cycle.
  * S^T computed into a [128,1024] 2-bank PSUM tile; ONE exp ACT per m-tile
    (N=1024) writing fp8 P^T directly in the DoubleRow pair layout.
  * exp(S*scale - 1.5) keeps fp8 range comfortable; the -1.5 cancels in the
    P/rowsum ratio.
  * GN rstd via ln+exp (exp(-0.5*ln(var+eps))) so the WHOLE kernel uses one
    ACT table set (natural_log_exp) -- no mid-kernel table reloads.
  * GN group broadcast via tiny PE matmuls instead of per-channel DMAs.
  * Fully fused emission: attention heads are interleaved ("zipped") with the
    other image's QKV / GN / proj work so the PE never waits on the ACT-bound
    softmax chain.
  * Both x tiles stay resident in SBUF for the residual (no re-DMA).
"""

import os
import numpy as np
import ml_dtypes

import concourse.bass as bass
import concourse.bacc as bacc
import concourse.tile as tile
from concourse import mybir
from concourse.bass_utils import run_bass_kernel_spmd

N_CORES = 8
B, C, HH, WW = 16, 512, 32, 32
N = HH * WW             # 1024 tokens
NH, DH = 4, 128
G, GS = 8, 64
B_LOC = B // N_CORES    # 2 images per core
EPS = 1e-5
CT = C // 128           # 4 channel tiles
NT = N // 128           # 8 token tiles
NCH = N // 512          # 2 free-dim chunks
NP = NT // 2            # 4 m-tile pairs
KP = CT // 2            # 2 kt pairs
SCALE = float(DH) ** -0.5
EXP_BIAS = -1.5

KMODE = os.environ.get("KMODE", "mixed")   # "mixed" | "fp8"
BITEXP = os.environ.get("BITEXP", "0") == "1"
LOG2E = 1.4426950408889634
BEXP_A = SCALE * LOG2E * (1 << 23)
BEXP_B = (127.0 - 1.5 * LOG2E - 0.043677) * (1 << 23)
FP8_QKV = KMODE == "fp8"

f32 = mybir.dt.float32
bf16 = mybir.dt.bfloat16
fp8 = mybir.dt.float8e4
QKV_D = fp8 if FP8_QKV else bf16

AF = mybir.ActivationFunctionType
OP = mybir.AluOpType
DR = mybir.MatmulPerfMode.DoubleRow

NP8 = ml_dtypes.float8_e4m3
NPBF = ml_dtypes.bfloat16


def build_program():
    nc = bacc.Bacc("TRN2", target_bir_lowering=False, debug=False)

    x_d = nc.dram_tensor("x", [B_LOC, C, N], f32, kind="ExternalInput").ap()
    if FP8_QKV:
        wqk_d = nc.dram_tensor("wqk", [KP, 128, 2, 2 * C], fp8,
                               kind="ExternalInput").ap()
        wv_d = nc.dram_tensor("wv", [KP, 128, 2, C], fp8,
                              kind="ExternalInput").ap()
    else:
        wqk_d = nc.dram_tensor("wqk", [CT, 128, 2 * C], bf16,
                               kind="ExternalInput").ap()
        wv_d = nc.dram_tensor("wv", [CT, 128, C], bf16,
                              kind="ExternalInput").ap()
    wp_d = nc.dram_tensor("wp", [KP, 128, 2, C], fp8, kind="ExternalInput").ap()
    qkb_d = nc.dram_tensor("qkb", [2 * C], f32, kind="ExternalInput").ap()
    vb_d = nc.dram_tensor("vb", [C], f32, kind="ExternalInput").ap()
    pb_d = nc.dram_tensor("pb", [C], f32, kind="ExternalInput").ap()
    gam_d = nc.dram_tensor("gamma", [C], f32, kind="ExternalInput").ap()
    bet_d = nc.dram_tensor("beta", [C], f32, kind="ExternalInput").ap()
    out_d = nc.dram_tensor("out", [B_LOC, C, N], f32, kind="ExternalOutput").ap()

    with tile.TileContext(nc) as tc:
        with (
            tc.tile_pool(name="wpool", bufs=1) as wpool,
            tc.tile_pool(name="xpool", bufs=1) as xpool,
            tc.tile_pool(name="xnpool", bufs=1) as xnpool,
            tc.tile_pool(name="qkpool", bufs=1) as qkpool,
            tc.tile_pool(name="vtpool", bufs=1) as vtpool,
            tc.tile_pool(name="otpool", bufs=1) as otpool,
            tc.tile_pool(name="ptpool", bufs=6) as ptpool,
            tc.tile_pool(name="oupool", bufs=2) as oupool,
            tc.tile_pool(name="rpool", bufs=2) as rpool,
            tc.tile_pool(name="outpool", bufs=2) as outpool,
            tc.tile_pool(name="spool", bufs=2) as spool,
            tc.tile_pool(name="mmps", bufs=2, space="PSUM") as mmps,
            tc.tile_pool(name="accps", bufs=1, space="PSUM") as accps,
        ):
            # ---------- small constants FIRST (the DMA queue is FIFO:
            # a tiny transfer behind megabytes of weights stalls the GN
            # broadcast matmuls and with them the whole in-order PE queue) ----
            sel = wpool.tile([128, 2], f32, tag="sel")
            nc.vector.memset(sel[0:64, 0:1], 1.0 / GS)
            nc.vector.memset(sel[64:128, 0:1], 0.0)
            nc.vector.memset(sel[0:64, 1:2], 0.0)
            nc.vector.memset(sel[64:128, 1:2], 1.0 / GS)
            # selB rows are 64-shifted windows of a [1,0,1] block pattern
            # (direct memset of partition row 1 is not a legal access).
            pat = wpool.tile([1, 192], f32, tag="selpat")
            nc.vector.memset(pat[0:1, 0:64], 1.0)
            nc.vector.memset(pat[0:1, 64:128], 0.0)
            nc.vector.memset(pat[0:1, 128:192], 1.0)
            selB = wpool.tile([2, 128], f32, tag="selB")
            nc.sync.dma_start(
                selB[:],
                bass.AP(tensor=pat.tensor, offset=pat.offset,
                        ap=[[1, 1], [64, 2], [1, 128]]))
            ones_f = wpool.tile([128, 2, 16], f32, tag="onesf")
            nc.vector.memset(ones_f[:], 1.0)
            ones8 = wpool.tile([128, 2, 16], fp8, tag="ones8")
            nc.vector.tensor_copy(ones8[:], ones_f[:])
            eps_t = wpool.tile([2, 1], f32, tag="eps")
            nc.vector.memset(eps_t[:], EPS)
            ebias = wpool.tile([128, 1], f32, tag="ebias")
            nc.vector.memset(ebias[:], EXP_BIAS)
            c_mh = wpool.tile([2, 1], f32, tag="cmh")
            nc.vector.memset(c_mh[:], -0.5)
            c_32 = wpool.tile([2, 1], f32, tag="c32")
            nc.vector.memset(c_32[:], 1.5)
            bA = wpool.tile([128, 1], f32, tag="bA")
            nc.vector.memset(bA[:], BEXP_A)
            bB = wpool.tile([128, 1], f32, tag="bB")
            nc.vector.memset(bB[:], BEXP_B)
            warm = wpool.tile([2, 1], f32, tag="warm")
            nc.vector.memset(warm[:], 1.0)
            wsc = spool.tile([2, 1], f32, tag="wsc", bufs=1)
            # preload the exp ACT table set during the DMA wait (the only
            # transcendental set this kernel uses)
            nc.scalar.activation(wsc[:], warm[:], AF.Exp)
            qkb_sb = wpool.tile([128, 2 * CT], f32, tag="qkb")
            nc.sync.dma_start(qkb_sb[:], qkb_d.rearrange("(t p) -> p t", p=128))
            pb_sb = wpool.tile([128, CT], f32, tag="pb")
            nc.sync.dma_start(pb_sb[:], pb_d.rearrange("(t p) -> p t", p=128))
            gam_sb = wpool.tile([128, CT], f32, tag="gam")
            nc.sync.dma_start(gam_sb[:], gam_d.rearrange("(t p) -> p t", p=128))
            bet_sb = wpool.tile([128, CT], f32, tag="bet")
            nc.sync.dma_start(bet_sb[:], bet_d.rearrange("(t p) -> p t", p=128))

            # ---------- input DMAs, priority order ----------
            xts = []
            for img in range(B_LOC):
                xt = xpool.tile([128, CT, N], f32, tag=f"x{img}", name=f"xt{img}")
                xts.append(xt)
            # x0 first (GN0 is the critical path)
            xr0 = x_d[0].rearrange("(t p) n -> p t n", p=128)
            for ct in range(CT):
                nc.sync.dma_start(xts[0][:, ct, :], xr0[:, ct, :])

            # qk weights (needed right after GN0)
            wqk_sb = []
            if FP8_QKV:
                for t in range(KP):
                    w = wpool.tile([128, 2, 2 * C], fp8, tag=f"wqk{t}",
                                   name=f"wqk{t}")
                    nc.sync.dma_start(w[:], wqk_d[t])
                    wqk_sb.append(w)
            else:
                for kt in range(CT):
                    w = wpool.tile([128, 2 * C], bf16, tag=f"wqk{kt}",
                                   name=f"wqk{kt}")
                    nc.sync.dma_start(w[:], wqk_d[kt])
                    wqk_sb.append(w)
            vb_bc = wpool.tile([128, C], f32, tag="vbbc")
            nc.sync.dma_start(
                vb_bc[:],
                bass.AP(tensor=vb_d.tensor, offset=vb_d.offset,
                        ap=[[0, 128], [1, C]]))
            wv_sb = []
            if FP8_QKV:
                for t in range(KP):
                    w = wpool.tile([128, 2, C], fp8, tag=f"wv{t}", name=f"wv{t}")
                    nc.sync.dma_start(w[:], wv_d[t])
                    wv_sb.append(w)
            else:
                for kt in range(CT):
                    w = wpool.tile([128, C], bf16, tag=f"wv{kt}", name=f"wv{kt}")
                    nc.sync.dma_start(w[:], wv_d[kt])
                    wv_sb.append(w)
            # x1 after qkv weights
            xr1 = x_d[1].rearrange("(t p) n -> p t n", p=128)
            for ct in range(CT):
                nc.sync.dma_start(xts[1][:, ct, :], xr1[:, ct, :])
            # proj weights last (needed only mid-kernel)
            wp_sb = []
            for t in range(KP):
                w = wpool.tile([128, 2, C], fp8, tag=f"wp{t}", name=f"wp{t}")
                nc.sync.dma_start(w[:], wp_d[t])
                wp_sb.append(w)

            xn_t = [None, None]
            qk_t = [None, None]
            vt_t = [None, None]
            ot_t = [None, None]

            # ---------- GroupNorm, per channel-tile ----------
            def gen_gn(img):
                """3 yields per ct so zipped PE work lands between the tiny
                GN matmul and its DVE-produced inputs (in-order PE queue:
                a stalled tiny MM stalls everything behind it)."""
                xn_t[img] = xnpool.tile([128, CT, N], QKV_D, tag=f"xn{img}",
                                        name=f"xn{img}")
                xt = xts[img]
                for ct in range(CT):
                    st = spool.tile([128, 2, 6], f32, tag="bnst", name="st")
                    nc.vector.bn_stats(st[:, 0, :], xt[:, ct, 0:512])
                    nc.vector.bn_stats(st[:, 1, :], xt[:, ct, 512:1024])
                    mv = spool.tile([128, 2], f32, tag="mv", name="mv")
                    nc.vector.bn_aggr(mv[:], st[:])
                    s2 = spool.tile([128, 2], f32, tag="s2", name="s2")
                    nc.vector.tensor_copy(s2[:, 0:1], mv[:, 0:1])
                    nc.vector.tensor_mul(s2[:, 1:2], mv[:, 0:1], mv[:, 0:1])
                    nc.vector.tensor_add(s2[:, 1:2], s2[:, 1:2], mv[:, 1:2])
                    yield
                    psg = accps.tile([2, 2], f32, tag="accr", name="psg", bufs=2)
                    nc.tensor.matmul(psg[:], sel[:], s2[:],
                                     start=True, stop=True)
                    gs = spool.tile([2, 2], f32, tag="gs", name="gs")
                    nc.vector.tensor_copy(gs[:], psg[:])
                    var_g = spool.tile([2, 1], f32, tag="gvar", name="var_g")
                    nc.vector.tensor_mul(var_g[:], gs[:, 0:1], gs[:, 0:1])
                    nc.vector.tensor_sub(var_g[:], gs[:, 1:2], var_g[:])
                    nc.vector.tensor_add(var_g[:], var_g[:], eps_t[:])
                    # rstd = 1/sqrt(v) via Newton from r0=1 (var ~= 1 by GN
                    # construction). Avoids Sqrt/Ln ACTs -> no table swaps.
                    r = spool.tile([2, 1], f32, tag="gnr", name="r")
                    nc.vector.tensor_scalar(
                        out=r[:], in0=var_g[:], scalar1=c_mh[:],
                        scalar2=c_32[:], op0=OP.mult, op1=OP.add)
                    t1 = spool.tile([2, 1], f32, tag="gnt", name="t1", bufs=4)
                    nc.vector.tensor_mul(t1[:], r[:], r[:])
                    nc.vector.tensor_mul(t1[:], t1[:], var_g[:])
                    nc.vector.tensor_scalar(
                        out=t1[:], in0=t1[:], scalar1=c_mh[:], scalar2=c_32[:],
                        op0=OP.mult, op1=OP.add)
                    nc.vector.tensor_mul(r[:], r[:], t1[:])
                    yield
                    a_ch = spool.tile([128, 1], f32, tag="ach", name="a_ch",
                                      bufs=4)
                    b_ch = spool.tile([128, 1], f32, tag="bch", name="b_ch",
                                      bufs=4)
                    if img == 0:
                        # broadcast group mu/rstd to channels via a tiny PE
                        # matmul (PE is idle during GN0)
                        mr = spool.tile([2, 2], f32, tag="mr", name="mr")
                        nc.vector.tensor_copy(mr[:, 0:1], gs[:, 0:1])
                        nc.vector.tensor_copy(mr[:, 1:2], r[:])
                        mubc = accps.tile([128, 2], f32, tag="accr", name="mubc", bufs=2)
                        nc.tensor.matmul(mubc[:], selB[:], mr[:],
                                         start=True, stop=True)
                        nc.vector.tensor_mul(a_ch[:], mubc[:, 1:2],
                                             gam_sb[:, ct:ct + 1])
                        nc.vector.tensor_mul(b_ch[:], mubc[:, 0:1], a_ch[:])
                    else:
                        # gn1 runs zipped inside attn0: broadcast via small
                        # DMAs so no PE instruction waits on this DVE chain
                        mu_ch = spool.tile([128, 1], f32, tag="much",
                                           name="mu_ch", bufs=4)
                        sg = gs[:, 0:1]
                        nc.sync.dma_start(
                            mu_ch[:],
                            bass.AP(tensor=sg.tensor, offset=sg.offset,
                                    ap=[[sg.ap[0][0], 2], [0, GS]]))
                        rs_ch = spool.tile([128, 1], f32, tag="rsch",
                                           name="rs_ch", bufs=4)
                        nc.sync.dma_start(
                            rs_ch[:],
                            bass.AP(tensor=r.tensor, offset=r.offset,
                                    ap=[[r.ap[0][0], 2], [0, GS]]))
                        nc.vector.tensor_mul(a_ch[:], rs_ch[:],
                                             gam_sb[:, ct:ct + 1])
                        nc.vector.tensor_mul(b_ch[:], mu_ch[:], a_ch[:])
                    nc.vector.tensor_sub(b_ch[:], bet_sb[:, ct:ct + 1],
                                         b_ch[:])
                    if img == 0:
                        # split the affine across DVE and ACT: halves the GN0
                        # critical path while ACT is otherwise idle
                        nc.vector.tensor_scalar(
                            out=xn_t[img][:, ct, 0:512], in0=xt[:, ct, 0:512],
                            scalar1=a_ch[:], scalar2=b_ch[:], op0=OP.mult,
                            op1=OP.add)
                        nc.scalar.activation(
                            xn_t[img][:, ct, 512:1024], xt[:, ct, 512:1024],
                            AF.Identity, scale=a_ch[:], bias=b_ch[:])
                    else:
                        nc.vector.tensor_scalar(
                            out=xn_t[img][:, ct, :], in0=xt[:, ct, :],
                            scalar1=a_ch[:], scalar2=b_ch[:], op0=OP.mult,
                            op1=OP.add)
                    yield

            # ---------- QKV: q,k channel-major ----------
            def qk_block(img, mt, on_act):
                """One output tile (o = mt*128..) of q|k: 1024-wide psum."""
                xn = xn_t[img]
                ps0 = accps.tile([128, 512], f32, tag="accr",
                                 name=f"qkps{img}_{mt}a", bufs=2)
                ps1 = accps.tile([128, 512], f32, tag="accr",
                                 name=f"qkps{img}_{mt}b", bufs=2)
                pss = [ps0, ps1]
                if FP8_QKV:
                    for t in range(KP):
                        for ch in range(NCH):
                            nc.tensor.matmul(
                                pss[ch][:],
                                wqk_sb[t][:, :, mt * 128:(mt + 1) * 128],
                                xn[:, 2 * t:2 * t + 2, ch * 512:(ch + 1) * 512],
                                start=(t == 0), stop=(t == KP - 1),
                                perf_mode=DR)
                else:
                    for kt in range(CT):
                        for ch in range(NCH):
                            nc.tensor.matmul(
                                pss[ch][:],
                                wqk_sb[kt][:, mt * 128:(mt + 1) * 128],
                                xn[:, kt, ch * 512:(ch + 1) * 512],
                                start=(kt == 0), stop=(kt == CT - 1))
                for ch in range(NCH):
                    if on_act:
                        nc.scalar.activation(
                            qk_t[img][:, mt, ch * 512:(ch + 1) * 512],
                            pss[ch][:], AF.Identity,
                            bias=qkb_sb[:, mt:mt + 1])
                    else:
                        nc.vector.tensor_scalar_add(
                            qk_t[img][:, mt, ch * 512:(ch + 1) * 512],
                            pss[ch][:], qkb_sb[:, mt:mt + 1])

            QK_ORDER = [0, 4, 1, 5, 2, 6, 3, 7]

            def gen_qk(img, mts, on_act):
                for mt in mts:
                    qk_block(img, mt, on_act)
                    yield

            # ---------- V: token-major fp8 ----------
            def v_block(img, nt):
                xn = xn_t[img]
                ps = accps.tile([128, C], f32, tag="accr", name=f"vps{img}_{nt}", bufs=2)
                if FP8_QKV:
                    for t in range(KP):
                        nc.tensor.matmul(
                            ps[:, 0:C],
                            xn[:, 2 * t:2 * t + 2, nt * 128:(nt + 1) * 128],
                            wv_sb[t][:],
                            start=(t == 0), stop=(t == KP - 1), perf_mode=DR)
                else:
                    for kt in range(CT):
                        nc.tensor.matmul(
                            ps[:, 0:C],
                            xn[:, kt, nt * 128:(nt + 1) * 128],
                            wv_sb[kt][:],
                            start=(kt == 0), stop=(kt == CT - 1))
                nc.vector.tensor_add(vt_t[img][:, nt, :], ps[:, 0:C], vb_bc[:])

            def gen_v(img, nts):
                for nt in nts:
                    v_block(img, nt)
                    yield

            # ---------- zip pump ----------
            from collections import deque
            zipq = deque()

            def pump(n):
                done = 0
                while zipq and done < n:
                    g = zipq[0]
                    try:
                        next(g)
                        done += 1
                    except StopIteration:
                        zipq.popleft()
                return done

            def drain():
                while zipq:
                    pump(1000)

            # ---------- attention ----------
            def attn_head(img, h, zip_per_pair, pending_finish=None):
                """Emit one head's S/exp/PV stream. The rowsum + normalize
                chain is returned as a closure and emitted inside the NEXT
                head's first pair, so its serial recip/broadcast latency never
                sits between two heads' exp streams."""
                qk = qk_t[img]
                vt = vt_t[img]
                acc0 = accps.tile([128, 512], f32, tag="acc0", name="acc0")
                acc1 = accps.tile([128, 512], f32, tag="acc1", name="acc1")
                accs = [acc0, acc1]
                pts = []
                for p in range(NP):
                    # pump BEFORE this pair's matmuls: zipped producers (e.g.
                    # v-blocks feeding PV) must precede their consumers in the
                    # PE stream
                    pump(zip_per_pair)
                    pt = ptpool.tile([128, 2, N], fp8, tag="pt", name=f"pt{p}")
                    pts.append(pt)
                    for j in range(2):
                        mt = 2 * p + j
                        sps = mmps.tile([128, N], f32, tag="mm",
                                        name=f"sps{h}_{mt}")
                        for ch in range(NCH):
                            nc.tensor.matmul(
                                sps[:, ch * 512:(ch + 1) * 512],
                                qk[:, NH + h, mt * 128:(mt + 1) * 128],
                                qk[:, h, ch * 512:(ch + 1) * 512],
                                start=True, stop=True)
                        if BITEXP and img == 1 and j == 0 and p % 2 == 1:
                            # piecewise-linear exp2 via int bitcast on DVE:
                            # offloads part of the exp stream from ACT
                            ti = oupool.tile([128, N], mybir.dt.int32,
                                             tag="bexp", name="ti", bufs=2)
                            nc.vector.tensor_scalar(
                                out=ti[:], in0=sps[:], scalar1=bA[:],
                                scalar2=bB[:], op0=OP.mult, op1=OP.add)
                            nc.vector.tensor_copy(
                                pt[:, j, :], ti[:].bitcast(f32))
                        else:
                            nc.scalar.activation(
                                pt[:, j, :], sps[:], AF.Exp,
                                scale=SCALE, bias=ebias[:])
                    if p == 0 and pending_finish is not None:
                        pending_finish()
                    for ch in range(NCH):
                        nc.tensor.matmul(
                            accs[ch][:],
                            vt[:, 2 * p:2 * p + 2, h * 128:(h + 1) * 128],
                            pt[:, :, ch * 512:(ch + 1) * 512],
                            start=(p == 0), stop=(p == NP - 1), perf_mode=DR)

                def finish():
                    # rowsum, one 512-chunk at a time (single PSUM bank)
                    rinv = rpool.tile([1, N], f32, tag="rinv", name="rinv")
                    for ch in range(NCH):
                        ps_r = accps.tile([1, 512], f32, tag="accr",
                                          name="ps_r", bufs=2)
                        for p in range(NP):
                            nc.tensor.matmul(
                                ps_r[:],
                                ones8[:, :, 0:1],
                                pts[p][:, :, ch * 512:(ch + 1) * 512],
                                start=(p == 0), stop=(p == NP - 1),
                                perf_mode=DR)
                        nc.vector.reciprocal_approx_fast(
                            rinv[:, ch * 512:(ch + 1) * 512], ps_r[:])
                    # evacuate PV psums (unnormalized) to free the banks
                    ot_u = oupool.tile([128, N], bf16, tag="otu", name="ot_u")
                    for ch in range(NCH):
                        nc.vector.tensor_copy(
                            ot_u[:, ch * 512:(ch + 1) * 512], accs[ch][:])
                    rb = rpool.tile([128, N], f32, tag="rb", name="rb")
                    for ch in range(NCH):
                        nc.gpsimd.partition_broadcast(
                            rb[:, ch * 512:(ch + 1) * 512],
                            rinv[:, ch * 512:(ch + 1) * 512], channels=128)
                    nc.vector.tensor_mul(ot_t[img][:, h, :], ot_u[:], rb[:])

                return finish

            def alloc_img(img):
                qk_t[img] = qkpool.tile([128, 2 * CT, N], QKV_D, tag=f"qk{img}",
                                        name=f"qk{img}")
                vt_t[img] = vtpool.tile([128, NT, C], fp8, tag=f"vt{img}",
                                        name=f"vt{img}")
                ot_t[img] = otpool.tile([128, NH, N], fp8, tag=f"ot{img}",
                                        name=f"ot{img}")

            # ---------- projection + residual ----------
            def proj_mm_group(img, pt_i, t, ps):
                ot = ot_t[img]
                for ch in range(NCH):
                    nc.tensor.matmul(
                        ps[ch][:],
                        wp_sb[t][:, :, pt_i * 128:(pt_i + 1) * 128],
                        ot[:, 2 * t:2 * t + 2, ch * 512:(ch + 1) * 512],
                        start=(t == 0), stop=(t == KP - 1), perf_mode=DR)

            def proj_mms(img, pt_i, zip_pool=False):
                if zip_pool:
                    psa = accps.tile([128, 512], f32, tag="accr",
                                     name=f"pps{img}_{pt_i}a", bufs=2)
                    psb = accps.tile([128, 512], f32, tag="accr",
                                     name=f"pps{img}_{pt_i}b", bufs=2)
                    ps = [psa, psb]
                else:
                    pst = mmps.tile([128, N], f32, tag="mm",
                                    name=f"pps{img}_{pt_i}")
                    ps = [pst[:, 0:512], pst[:, 512:1024]]
                proj_mm_group(img, pt_i, 0, ps)
                return ps

            def proj_fin(img, pt_i, ps):
                proj_mm_group(img, pt_i, 1, ps)
                outt = outpool.tile([128, N], f32, tag="outt",
                                    name=f"o{img}_{pt_i}")
                for ch in range(NCH):
                    nc.vector.scalar_tensor_tensor(
                        out=outt[:, ch * 512:(ch + 1) * 512],
                        in0=ps[ch][:],
                        scalar=pb_sb[:, pt_i:pt_i + 1],
                        in1=xts[img][:, pt_i, ch * 512:(ch + 1) * 512],
                        op0=OP.add, op1=OP.add)
                for ch in range(NCH):
                    nc.sync.dma_start(
                        out_d[img, pt_i * 128:(pt_i + 1) * 128,
                              ch * 512:(ch + 1) * 512],
                        outt[:, ch * 512:(ch + 1) * 512])

            def proj_block(img, pt_i):
                proj_fin(img, pt_i, proj_mms(img, pt_i, zip_pool=(img == 0)))

            def gen_proj(img):
                for pt_i in range(CT):
                    proj_block(img, pt_i)
                    yield

            def gn0_batched():
                """GN for img0 with all 4 cts' group math vectorized: the
                ~350ns/op in-order DVE queue cost makes per-ct serial chains
                the startup pacer."""
                img = 0
                xn_t[img] = xnpool.tile([128, CT, N], QKV_D, tag=f"xn{img}",
                                        name=f"xn{img}")
                xt = xts[img]
                s2a = spool.tile([128, 2 * CT], f32, tag="s2a", bufs=1)
                for ct in range(CT):
                    st = spool.tile([128, 2, 6], f32, tag="bnst", name="st")
                    nc.vector.bn_stats(st[:, 0, :], xt[:, ct, 0:512])
                    nc.vector.bn_stats(st[:, 1, :], xt[:, ct, 512:1024])
                    mv = spool.tile([128, 2], f32, tag="mv", name="mv")
                    nc.vector.bn_aggr(mv[:], st[:])
                    nc.vector.tensor_copy(s2a[:, 2 * ct:2 * ct + 1],
                                          mv[:, 0:1])
                    nc.vector.tensor_mul(s2a[:, 2 * ct + 1:2 * ct + 2],
                                         mv[:, 0:1], mv[:, 0:1])
                    nc.vector.tensor_add(s2a[:, 2 * ct + 1:2 * ct + 2],
                                         s2a[:, 2 * ct + 1:2 * ct + 2],
                                         mv[:, 1:2])
                psg = accps.tile([2, 2 * CT], f32, tag="accr", name="psg0", bufs=2)
                nc.tensor.matmul(psg[:], sel[:], s2a[:], start=True, stop=True)
                gs = spool.tile([2, 2 * CT], f32, tag="gs0", bufs=1)
                nc.vector.tensor_copy(gs[:], psg[:])
                gs3 = gs[:].rearrange("p (t s) -> p t s", s=2)
                var_g = spool.tile([2, CT], f32, tag="gvar0", bufs=1)
                nc.vector.tensor_mul(var_g[:], gs3[:, :, 0], gs3[:, :, 0])
                nc.vector.tensor_sub(var_g[:], gs3[:, :, 1], var_g[:])
                nc.vector.tensor_scalar(
                    out=var_g[:], in0=var_g[:], scalar1=eps_t[:], scalar2=None,
                    op0=OP.add)
                r = spool.tile([2, CT], f32, tag="gnr0", bufs=1)
                nc.vector.tensor_scalar(
                    out=r[:], in0=var_g[:], scalar1=c_mh[:], scalar2=c_32[:],
                    op0=OP.mult, op1=OP.add)
                t1 = spool.tile([2, CT], f32, tag="gnt0", bufs=1)
                nc.vector.tensor_mul(t1[:], r[:], r[:])
                nc.vector.tensor_mul(t1[:], t1[:], var_g[:])
                nc.vector.tensor_scalar(
                    out=t1[:], in0=t1[:], scalar1=c_mh[:], scalar2=c_32[:],
                    op0=OP.mult, op1=OP.add)
                nc.vector.tensor_mul(r[:], r[:], t1[:])
                mr = spool.tile([2, 2 * CT], f32, tag="mr0", bufs=1)
                mr3 = mr[:].rearrange("p (t s) -> p t s", s=2)
                nc.vector.tensor_copy(mr3[:, :, 0], gs3[:, :, 0])
                nc.vector.tensor_copy(mr3[:, :, 1], r[:])
                mubc = accps.tile([128, 2 * CT], f32, tag="accr", name="mubc0", bufs=2)
                nc.tensor.matmul(mubc[:], selB[:], mr[:], start=True,
                                 stop=True)
                mu3 = mubc[:].rearrange("p (t s) -> p t s", s=2)
                a_a = spool.tile([128, CT], f32, tag="aa0", bufs=1)
                nc.vector.tensor_mul(a_a[:], mu3[:, :, 1], gam_sb[:])
                b_a = spool.tile([128, CT], f32, tag="ba0", bufs=1)
                nc.vector.tensor_mul(b_a[:], mu3[:, :, 0], a_a[:])
                nc.vector.tensor_sub(b_a[:], bet_sb[:], b_a[:])
                for ct in range(CT):
                    nc.vector.tensor_scalar(
                        out=xn_t[img][:, ct, 0:512], in0=xt[:, ct, 0:512],
                        scalar1=a_a[:, ct:ct + 1], scalar2=b_a[:, ct:ct + 1],
                        op0=OP.mult, op1=OP.add)
                    nc.scalar.activation(
                        xn_t[img][:, ct, 512:1024], xt[:, ct, 512:1024],
                        AF.Identity, scale=a_a[:, ct:ct + 1],
                        bias=b_a[:, ct:ct + 1])

            # ================= emission schedule =================
            alloc_img(0)
            alloc_img(1)
            # GN0 inline (critical path), batched
            gn0_batched()
            # head0 prerequisites: q,k slots 0 and 4 (v0 is zipped)
            qk_block(0, 0, on_act=True)
            qk_block(0, 4, on_act=True)
            # zip queue: img0 v + rest of img0 qk, then gn1 + img1 qkv.
            # Zipped qk evacuations ALTERNATE between ACT and DVE so neither
            # in-order queue stalls the exp stream or the S-matmul inputs.
            zipq.append(gen_v(0, range(NT)))
            zipq.append(gen_qk(0, [1, 5], on_act=True))
            zipq.append(gen_qk(0, [2, 6], on_act=False))
            zipq.append(gen_gn(1))
            zipq.append(gen_qk(0, [3, 7], on_act=True))
            zipq.append(gen_qk(1, [0, 4], on_act=False))
            zipq.append(gen_v(1, range(0, 6)))
            zipq.append(gen_qk(1, [1, 5], on_act=True))
            for h in range(NH):
                attn_head(0, h, zip_per_pair=3)()
            # attn1: next heads' qk first, then v tail, proj0, last head's qk
            zipq.append(gen_qk(1, [2, 6], on_act=False))
            zipq.append(gen_v(1, range(6, NT)))
            zipq.append(gen_proj(0))
            zipq.append(gen_qk(1, [3, 7], on_act=True))
            for h in range(NH):
                attn_head(1, h, zip_per_pair=2)()
            drain()
            # tail: interleave proj1 blocks pairwise -- the first
            # accumulation step (head-pair 0) doesn't depend on the last
            # head's normalize chain, so it runs while that chain drains
            for b0, b1 in [(0, 1), (2, 3)]:
                ps0 = proj_mms(1, b0)
                ps1 = proj_mms(1, b1)
                proj_fin(1, b0, ps0)
                proj_fin(1, b1, ps1)

    nc.compile()
    return nc


_NC_CACHE = None


def _get_nc():
    global _NC_CACHE
    if _NC_CACHE is None:
        _NC_CACHE = build_program()
    return _NC_CACHE


def _host_prep(x, norm_gamma, norm_beta, qkv_w, qkv_b, proj_w, proj_b):
    qkv_w = np.ascontiguousarray(qkv_w, dtype=np.float32)
    proj_w = np.ascontiguousarray(proj_w, dtype=np.float32)
    wqkT = qkv_w[:2 * C].T          # [c, o] = [512, 1024]
    wvT = qkv_w[2 * C:].T           # [512, 512]
    wpT = proj_w.T                  # [512, 512]
    if FP8_QKV:
        wqk = np.ascontiguousarray(
            wqkT.reshape(KP, 2, 128, 2 * C).transpose(0, 2, 1, 3)).astype(NP8)
        wv = np.ascontiguousarray(
            wvT.reshape(KP, 2, 128, C).transpose(0, 2, 1, 3)).astype(NP8)
    else:
        wqk = np.ascontiguousarray(wqkT.reshape(CT, 128, 2 * C)).astype(NPBF)
        wv = np.ascontiguousarray(wvT.reshape(CT, 128, C)).astype(NPBF)
    wp = np.ascontiguousarray(
        wpT.reshape(KP, 2, 128, C).transpose(0, 2, 1, 3)).astype(NP8)
    common = {
        "wqk": wqk, "wv": wv, "wp": wp,
        "qkb": np.ascontiguousarray(qkv_b[:2 * C], dtype=np.float32),
        "vb": np.ascontiguousarray(qkv_b[2 * C:], dtype=np.float32),
        "pb": np.ascontiguousarray(proj_b, dtype=np.float32),
        "gamma": np.ascontiguousarray(norm_gamma, dtype=np.float32),
        "beta": np.ascontiguousarray(norm_beta, dtype=np.float32),
    }
    xr = np.ascontiguousarray(np.asarray(x, dtype=np.float32).reshape(B, C, N))
    in_maps = []
    for c in range(N_CORES):
        m = dict(common)
        m["x"] = np.ascontiguousarray(xr[c * B_LOC:(c + 1) * B_LOC])
        in_maps.append(m)
    return in_maps


def run(inputs, trace=False):
    nc = _get_nc()
    in_maps = _host_prep(**inputs)
    res = None
    for attempt in range(3):
        try:
            res = run_bass_kernel_spmd(
                nc, in_maps, core_ids=list(range(N_CORES)), trace=trace)
            break
        except Exception:
            if attempt == 2:
                raise
    parts = [res.results[c]["out"] for c in range(N_CORES)]
    out = np.concatenate(parts, axis=0).reshape(B, C, HH, WW)
    return out.astype(np.float32), res


def kernel(**inputs):
    out, _ = run(inputs, trace=False)
    return out



# revision 2
# speedup vs baseline: 1.0574x; 1.0574x over previous
"""Trainium2 Bass kernel v2 for nn_AttentionBlock (GroupNorm + MHA + proj + residual).

x: [16, 512, 32, 32] fp32, 8 cores data-parallel (2 images/core).

Key differences vs v1:
  * fp8(e4m3) + DoubleRow matmuls for PV, softmax-rowsum and proj (and
    optionally the QKV convs with KMODE=fp8): 2x contraction per cycle.
  * S^T computed into a [128,1024] 2-bank PSUM tile; ONE exp ACT per m-tile
    (N=1024) writing fp8 P^T directly in the DoubleRow pair layout.
  * exp(S*scale - 1.5) keeps fp8 range comfortable; the -1.5 cancels in the
    P/rowsum ratio.
  * GN rstd via ln+exp (exp(-0.5*ln(var+eps))) so the WHOLE kernel uses one
    ACT table set (natural_log_exp) -- no mid-kernel table reloads.
  * GN group broadcast via tiny PE matmuls instead of per-channel DMAs.
  * Fully fused emission: attention heads are interleaved ("zipped") with the
    other image's QKV / GN / proj work so the PE never waits on the ACT-bound
    softmax chain.
  * Both x tiles stay resident in SBUF for the residual (no re-DMA).
"""

import os
import numpy as np
import ml_dtypes

import concourse.bass as bass
import concourse.bacc as bacc
import concourse.tile as tile
from concourse import mybir
from concourse.bass_utils import run_bass_kernel_spmd

N_CORES = 8
B, C, HH, WW = 16, 512, 32, 32
N = HH * WW             # 1024 tokens
NH, DH = 4, 128
G, GS = 8, 64
B_LOC = B // N_CORES    # 2 images per core
EPS = 1e-5
CT = C // 128           # 4 channel tiles
NT = N // 128           # 8 token tiles
NCH = N // 512          # 2 free-dim chunks
NP = NT // 2            # 4 m-tile pairs
KP = CT // 2            # 2 kt pairs
SCALE = float(DH) ** -0.5
EXP_BIAS = -1.5

KMODE = os.environ.get("KMODE", "fp8")   # "mixed" | "fp8"
BITEXP = os.environ.get("BITEXP", "0") == "1"
LOG2E = 1.4426950408889634
BEXP_A = SCALE * LOG2E * (1 << 23)
BEXP_B = (127.0 - 1.5 * LOG2E - 0.043677) * (1 << 23)
FP8_QKV = KMODE == "fp8"

f32 = mybir.dt.float32
bf16 = mybir.dt.bfloat16
fp8 = mybir.dt.float8e4
QKV_D = fp8 if FP8_QKV else bf16

AF = mybir.ActivationFunctionType
OP = mybir.AluOpType
DR = mybir.MatmulPerfMode.DoubleRow

NP8 = ml_dtypes.float8_e4m3
NPBF = ml_dtypes.bfloat16


def build_program():
    nc = bacc.Bacc("TRN2", target_bir_lowering=False, debug=False)

    x_d = nc.dram_tensor("x", [B_LOC, C, N], f32, kind="ExternalInput").ap()
    if FP8_QKV:
        wqk_d = nc.dram_tensor("wqk", [KP, 128, 2, 2 * C], fp8,
                               kind="ExternalInput").ap()
        wv_d = nc.dram_tensor("wv", [KP, 128, 2, C], fp8,
                              kind="ExternalInput").ap()
    else:
        wqk_d = nc.dram_tensor("wqk", [CT, 128, 2 * C], bf16,
                               kind="ExternalInput").ap()
        wv_d = nc.dram_tensor("wv", [CT, 128, C], bf16,
                              kind="ExternalInput").ap()
    wp_d = nc.dram_tensor("wp", [KP, 128, 2, C], fp8, kind="ExternalInput").ap()
    qkb_d = nc.dram_tensor("qkb", [2 * C], f32, kind="ExternalInput").ap()
    vb_d = nc.dram_tensor("vb", [C], f32, kind="ExternalInput").ap()
    pb_d = nc.dram_tensor("pb", [C], f32, kind="ExternalInput").ap()
    gam_d = nc.dram_tensor("gamma", [C], f32, kind="ExternalInput").ap()
    bet_d = nc.dram_tensor("beta", [C], f32, kind="ExternalInput").ap()
    out_d = nc.dram_tensor("out", [B_LOC, C, N], f32, kind="ExternalOutput").ap()

    with tile.TileContext(nc) as tc:
        with (
            tc.tile_pool(name="wpool", bufs=1) as wpool,
            tc.tile_pool(name="xpool", bufs=1) as xpool,
            tc.tile_pool(name="xnpool", bufs=1) as xnpool,
            tc.tile_pool(name="qkpool", bufs=1) as qkpool,
            tc.tile_pool(name="vtpool", bufs=1) as vtpool,
            tc.tile_pool(name="otpool", bufs=1) as otpool,
            tc.tile_pool(name="ptpool", bufs=6) as ptpool,
            tc.tile_pool(name="oupool", bufs=2) as oupool,
            tc.tile_pool(name="rpool", bufs=2) as rpool,
            tc.tile_pool(name="outpool", bufs=2) as outpool,
            tc.tile_pool(name="spool", bufs=2) as spool,
            tc.tile_pool(name="mmps", bufs=2, space="PSUM") as mmps,
            tc.tile_pool(name="accps", bufs=1, space="PSUM") as accps,
        ):
            # ---------- small constants FIRST (the DMA queue is FIFO:
            # a tiny transfer behind megabytes of weights stalls the GN
            # broadcast matmuls and with them the whole in-order PE queue) ----
            sel = wpool.tile([128, 2], f32, tag="sel")
            nc.vector.memset(sel[0:64, 0:1], 1.0 / GS)
            nc.vector.memset(sel[64:128, 0:1], 0.0)
            nc.vector.memset(sel[0:64, 1:2], 0.0)
            nc.vector.memset(sel[64:128, 1:2], 1.0 / GS)
            # selB rows are 64-shifted windows of a [1,0,1] block pattern
            # (direct memset of partition row 1 is not a legal access).
            pat = wpool.tile([1, 192], f32, tag="selpat")
            nc.vector.memset(pat[0:1, 0:64], 1.0)
            nc.vector.memset(pat[0:1, 64:128], 0.0)
            nc.vector.memset(pat[0:1, 128:192], 1.0)
            selB = wpool.tile([2, 128], f32, tag="selB")
            nc.sync.dma_start(
                selB[:],
                bass.AP(tensor=pat.tensor, offset=pat.offset,
                        ap=[[1, 1], [64, 2], [1, 128]]))
            ones_f = wpool.tile([128, 2, 16], f32, tag="onesf")
            nc.vector.memset(ones_f[:], 1.0)
            ones8 = wpool.tile([128, 2, 16], fp8, tag="ones8")
            nc.vector.tensor_copy(ones8[:], ones_f[:])
            eps_t = wpool.tile([2, 1], f32, tag="eps")
            nc.vector.memset(eps_t[:], EPS)
            ebias = wpool.tile([128, 1], f32, tag="ebias")
            nc.vector.memset(ebias[:], EXP_BIAS)
            c_mh = wpool.tile([2, 1], f32, tag="cmh")
            nc.vector.memset(c_mh[:], -0.5)
            c_32 = wpool.tile([2, 1], f32, tag="c32")
            nc.vector.memset(c_32[:], 1.5)
            bA = wpool.tile([128, 1], f32, tag="bA")
            nc.vector.memset(bA[:], BEXP_A)
            bB = wpool.tile([128, 1], f32, tag="bB")
            nc.vector.memset(bB[:], BEXP_B)
            warm = wpool.tile([2, 1], f32, tag="warm")
            nc.vector.memset(warm[:], 1.0)
            wsc = spool.tile([2, 1], f32, tag="wsc", bufs=1)
            # preload the exp ACT table set during the DMA wait (the only
            # transcendental set this kernel uses)
            nc.scalar.activation(wsc[:], warm[:], AF.Exp)
            qkb_sb = wpool.tile([128, 2 * CT], f32, tag="qkb")
            nc.sync.dma_start(qkb_sb[:], qkb_d.rearrange("(t p) -> p t", p=128))
            pb_sb = wpool.tile([128, CT], f32, tag="pb")
            nc.sync.dma_start(pb_sb[:], pb_d.rearrange("(t p) -> p t", p=128))
            gam_sb = wpool.tile([128, CT], f32, tag="gam")
            nc.sync.dma_start(gam_sb[:], gam_d.rearrange("(t p) -> p t", p=128))
            bet_sb = wpool.tile([128, CT], f32, tag="bet")
            nc.sync.dma_start(bet_sb[:], bet_d.rearrange("(t p) -> p t", p=128))

            # ---------- input DMAs, priority order ----------
            xts = []
            for img in range(B_LOC):
                xt = xpool.tile([128, CT, N], f32, tag=f"x{img}", name=f"xt{img}")
                xts.append(xt)
            # x0 first (GN0 is the critical path)
            xr0 = x_d[0].rearrange("(t p) n -> p t n", p=128)
            for ct in range(CT):
                nc.sync.dma_start(xts[0][:, ct, :], xr0[:, ct, :])

            # qk weights (needed right after GN0)
            wqk_sb = []
            if FP8_QKV:
                for t in range(KP):
                    w = wpool.tile([128, 2, 2 * C], fp8, tag=f"wqk{t}",
                                   name=f"wqk{t}")
                    nc.sync.dma_start(w[:], wqk_d[t])
                    wqk_sb.append(w)
            else:
                for kt in range(CT):
                    w = wpool.tile([128, 2 * C], bf16, tag=f"wqk{kt}",
                                   name=f"wqk{kt}")
                    nc.sync.dma_start(w[:], wqk_d[kt])
                    wqk_sb.append(w)
            vb_bc = wpool.tile([128, C], f32, tag="vbbc")
            nc.sync.dma_start(
                vb_bc[:],
                bass.AP(tensor=vb_d.tensor, offset=vb_d.offset,
                        ap=[[0, 128], [1, C]]))
            wv_sb = []
            if FP8_QKV:
                for t in range(KP):
                    w = wpool.tile([128, 2, C], fp8, tag=f"wv{t}", name=f"wv{t}")
                    nc.sync.dma_start(w[:], wv_d[t])
                    wv_sb.append(w)
            else:
                for kt in range(CT):
                    w = wpool.tile([128, C], bf16, tag=f"wv{kt}", name=f"wv{kt}")
                    nc.sync.dma_start(w[:], wv_d[kt])
                    wv_sb.append(w)
            # x1 after qkv weights
            xr1 = x_d[1].rearrange("(t p) n -> p t n", p=128)
            for ct in range(CT):
                nc.sync.dma_start(xts[1][:, ct, :], xr1[:, ct, :])
            # proj weights last (needed only mid-kernel)
            wp_sb = []
            for t in range(KP):
                w = wpool.tile([128, 2, C], fp8, tag=f"wp{t}", name=f"wp{t}")
                nc.sync.dma_start(w[:], wp_d[t])
                wp_sb.append(w)

            xn_t = [None, None]
            qk_t = [None, None]
            vt_t = [None, None]
            ot_t = [None, None]

            # ---------- GroupNorm, per channel-tile ----------
            def gen_gn(img):
                """3 yields per ct so zipped PE work lands between the tiny
                GN matmul and its DVE-produced inputs (in-order PE queue:
                a stalled tiny MM stalls everything behind it)."""
                xn_t[img] = xnpool.tile([128, CT, N], QKV_D, tag=f"xn{img}",
                                        name=f"xn{img}")
                xt = xts[img]
                for ct in range(CT):
                    st = spool.tile([128, 2, 6], f32, tag="bnst", name="st")
                    nc.vector.bn_stats(st[:, 0, :], xt[:, ct, 0:512])
                    nc.vector.bn_stats(st[:, 1, :], xt[:, ct, 512:1024])
                    mv = spool.tile([128, 2], f32, tag="mv", name="mv")
                    nc.vector.bn_aggr(mv[:], st[:])
                    s2 = spool.tile([128, 2], f32, tag="s2", name="s2")
                    nc.vector.tensor_copy(s2[:, 0:1], mv[:, 0:1])
                    nc.vector.tensor_mul(s2[:, 1:2], mv[:, 0:1], mv[:, 0:1])
                    nc.vector.tensor_add(s2[:, 1:2], s2[:, 1:2], mv[:, 1:2])
                    yield
                    psg = accps.tile([2, 2], f32, tag="accr", name="psg", bufs=2)
                    nc.tensor.matmul(psg[:], sel[:], s2[:],
                                     start=True, stop=True)
                    gs = spool.tile([2, 2], f32, tag="gs", name="gs")
                    nc.vector.tensor_copy(gs[:], psg[:])
                    var_g = spool.tile([2, 1], f32, tag="gvar", name="var_g")
                    nc.vector.tensor_mul(var_g[:], gs[:, 0:1], gs[:, 0:1])
                    nc.vector.tensor_sub(var_g[:], gs[:, 1:2], var_g[:])
                    nc.vector.tensor_add(var_g[:], var_g[:], eps_t[:])
                    # rstd = 1/sqrt(v) via Newton from r0=1 (var ~= 1 by GN
                    # construction). Avoids Sqrt/Ln ACTs -> no table swaps.
                    r = spool.tile([2, 1], f32, tag="gnr", name="r")
                    nc.vector.tensor_scalar(
                        out=r[:], in0=var_g[:], scalar1=c_mh[:],
                        scalar2=c_32[:], op0=OP.mult, op1=OP.add)
                    t1 = spool.tile([2, 1], f32, tag="gnt", name="t1", bufs=4)
                    nc.vector.tensor_mul(t1[:], r[:], r[:])
                    nc.vector.tensor_mul(t1[:], t1[:], var_g[:])
                    nc.vector.tensor_scalar(
                        out=t1[:], in0=t1[:], scalar1=c_mh[:], scalar2=c_32[:],
                        op0=OP.mult, op1=OP.add)
                    nc.vector.tensor_mul(r[:], r[:], t1[:])
                    yield
                    a_ch = spool.tile([128, 1], f32, tag="ach", name="a_ch",
                                      bufs=4)
                    b_ch = spool.tile([128, 1], f32, tag="bch", name="b_ch",
                                      bufs=4)
                    if img == 0:
                        # broadcast group mu/rstd to channels via a tiny PE
                        # matmul (PE is idle during GN0)
                        mr = spool.tile([2, 2], f32, tag="mr", name="mr")
                        nc.vector.tensor_copy(mr[:, 0:1], gs[:, 0:1])
                        nc.vector.tensor_copy(mr[:, 1:2], r[:])
                        mubc = accps.tile([128, 2], f32, tag="accr", name="mubc", bufs=2)
                        nc.tensor.matmul(mubc[:], selB[:], mr[:],
                                         start=True, stop=True)
                        nc.vector.tensor_mul(a_ch[:], mubc[:, 1:2],
                                             gam_sb[:, ct:ct + 1])
                        nc.vector.tensor_mul(b_ch[:], mubc[:, 0:1], a_ch[:])
                    else:
                        # gn1 runs zipped inside attn0: broadcast via small
                        # DMAs so no PE instruction waits on this DVE chain
                        mu_ch = spool.tile([128, 1], f32, tag="much",
                                           name="mu_ch", bufs=4)
                        sg = gs[:, 0:1]
                        nc.sync.dma_start(
                            mu_ch[:],
                            bass.AP(tensor=sg.tensor, offset=sg.offset,
                                    ap=[[sg.ap[0][0], 2], [0, GS]]))
                        rs_ch = spool.tile([128, 1], f32, tag="rsch",
                                           name="rs_ch", bufs=4)
                        nc.sync.dma_start(
                            rs_ch[:],
                            bass.AP(tensor=r.tensor, offset=r.offset,
                                    ap=[[r.ap[0][0], 2], [0, GS]]))
                        nc.vector.tensor_mul(a_ch[:], rs_ch[:],
                                             gam_sb[:, ct:ct + 1])
                        nc.vector.tensor_mul(b_ch[:], mu_ch[:], a_ch[:])
                    nc.vector.tensor_sub(b_ch[:], bet_sb[:, ct:ct + 1],
                                         b_ch[:])
                    if img == 0:
                        # split the affine across DVE and ACT: halves the GN0
                        # critical path while ACT is otherwise idle
                        nc.vector.tensor_scalar(
                            out=xn_t[img][:, ct, 0:512], in0=xt[:, ct, 0:512],
                            scalar1=a_ch[:], scalar2=b_ch[:], op0=OP.mult,
                            op1=OP.add)
                        nc.scalar.activation(
                            xn_t[img][:, ct, 512:1024], xt[:, ct, 512:1024],
                            AF.Identity, scale=a_ch[:], bias=b_ch[:])
                    else:
                        nc.vector.tensor_scalar(
                            out=xn_t[img][:, ct, :], in0=xt[:, ct, :],
                            scalar1=a_ch[:], scalar2=b_ch[:], op0=OP.mult,
                            op1=OP.add)
                    yield

            # ---------- QKV: q,k channel-major ----------
            def qk_block(img, mt, on_act):
                """One output tile (o = mt*128..) of q|k: 1024-wide psum."""
                xn = xn_t[img]
                ps0 = accps.tile([128, 512], f32, tag="accr",
                                 name=f"qkps{img}_{mt}a", bufs=2)
                ps1 = accps.tile([128, 512], f32, tag="accr",
                                 name=f"qkps{img}_{mt}b", bufs=2)
                pss = [ps0, ps1]
                if FP8_QKV:
                    for t in range(KP):
                        for ch in range(NCH):
                            nc.tensor.matmul(
                                pss[ch][:],
                                wqk_sb[t][:, :, mt * 128:(mt + 1) * 128],
                                xn[:, 2 * t:2 * t + 2, ch * 512:(ch + 1) * 512],
                                start=(t == 0), stop=(t == KP - 1),
                                perf_mode=DR)
                else:
                    for kt in range(CT):
                        for ch in range(NCH):
                            nc.tensor.matmul(
                                pss[ch][:],
                                wqk_sb[kt][:, mt * 128:(mt + 1) * 128],
                                xn[:, kt, ch * 512:(ch + 1) * 512],
                                start=(kt == 0), stop=(kt == CT - 1))
                for ch in range(NCH):
                    if on_act:
                        nc.scalar.activation(
                            qk_t[img][:, mt, ch * 512:(ch + 1) * 512],
                            pss[ch][:], AF.Identity,
                            bias=qkb_sb[:, mt:mt + 1])
                    else:
                        nc.vector.tensor_scalar_add(
                            qk_t[img][:, mt, ch * 512:(ch + 1) * 512],
                            pss[ch][:], qkb_sb[:, mt:mt + 1])

            QK_ORDER = [0, 4, 1, 5, 2, 6, 3, 7]

            def gen_qk(img, mts, on_act):
                for mt in mts:
                    qk_block(img, mt, on_act)
                    yield

            # ---------- V: token-major fp8 ----------
            def v_block(img, nt):
                xn = xn_t[img]
                ps = accps.tile([128, C], f32, tag="accr", name=f"vps{img}_{nt}", bufs=2)
                if FP8_QKV:
                    for t in range(KP):
                        nc.tensor.matmul(
                            ps[:, 0:C],
                            xn[:, 2 * t:2 * t + 2, nt * 128:(nt + 1) * 128],
                            wv_sb[t][:],
                            start=(t == 0), stop=(t == KP - 1), perf_mode=DR)
                else:
                    for kt in range(CT):
                        nc.tensor.matmul(
                            ps[:, 0:C],
                            xn[:, kt, nt * 128:(nt + 1) * 128],
                            wv_sb[kt][:],
                            start=(kt == 0), stop=(kt == CT - 1))
                nc.vector.tensor_add(vt_t[img][:, nt, :], ps[:, 0:C], vb_bc[:])

            def gen_v(img, nts):
                for nt in nts:
                    v_block(img, nt)
                    yield

            # ---------- zip pump ----------
            from collections import deque
            zipq = deque()

            def pump(n):
                done = 0
                while zipq and done < n:
                    g = zipq[0]
                    try:
                        next(g)
                        done += 1
                    except StopIteration:
                        zipq.popleft()
                return done

            def drain():
                while zipq:
                    pump(1000)

            # ---------- attention ----------
            def attn_head(img, h, zip_per_pair, pending_finish=None):
                """Emit one head's S/exp/PV stream. The rowsum + normalize
                chain is returned as a closure and emitted inside the NEXT
                head's first pair, so its serial recip/broadcast latency never
                sits between two heads' exp streams."""
                qk = qk_t[img]
                vt = vt_t[img]
                acc0 = accps.tile([128, 512], f32, tag="acc0", name="acc0")
                acc1 = accps.tile([128, 512], f32, tag="acc1", name="acc1")
                accs = [acc0, acc1]
                pts = []
                for p in range(NP):
                    # pump BEFORE this pair's matmuls: zipped producers (e.g.
                    # v-blocks feeding PV) must precede their consumers in the
                    # PE stream
                    pump(zip_per_pair)
                    pt = ptpool.tile([128, 2, N], fp8, tag="pt", name=f"pt{p}")
                    pts.append(pt)
                    for j in range(2):
                        mt = 2 * p + j
                        sps = mmps.tile([128, N], f32, tag="mm",
                                        name=f"sps{h}_{mt}")
                        for ch in range(NCH):
                            nc.tensor.matmul(
                                sps[:, ch * 512:(ch + 1) * 512],
                                qk[:, NH + h, mt * 128:(mt + 1) * 128],
                                qk[:, h, ch * 512:(ch + 1) * 512],
                                start=True, stop=True)
                        if BITEXP and img == 1 and j == 0 and p % 2 == 1:
                            # piecewise-linear exp2 via int bitcast on DVE:
                            # offloads part of the exp stream from ACT
                            ti = oupool.tile([128, N], mybir.dt.int32,
                                             tag="bexp", name="ti", bufs=2)
                            nc.vector.tensor_scalar(
                                out=ti[:], in0=sps[:], scalar1=bA[:],
                                scalar2=bB[:], op0=OP.mult, op1=OP.add)
                            nc.vector.tensor_copy(
                                pt[:, j, :], ti[:].bitcast(f32))
                        else:
                            nc.scalar.activation(
                                pt[:, j, :], sps[:], AF.Exp,
                                scale=SCALE, bias=ebias[:])
                    if p == 0 and pending_finish is not None:
                        pending_finish()
                    for ch in range(NCH):
                        nc.tensor.matmul(
                            accs[ch][:],
                            vt[:, 2 * p:2 * p + 2, h * 128:(h + 1) * 128],
                            pt[:, :, ch * 512:(ch + 1) * 512],
                            start=(p == 0), stop=(p == NP - 1), perf_mode=DR)

                def finish():
                    # rowsum, one 512-chunk at a time (single PSUM bank)
                    rinv = rpool.tile([1, N], f32, tag="rinv", name="rinv")
                    for ch in range(NCH):
                        ps_r = accps.tile([1, 512], f32, tag="accr",
                                          name="ps_r", bufs=2)
                        for p in range(NP):
                            nc.tensor.matmul(
                                ps_r[:],
                                ones8[:, :, 0:1],
                                pts[p][:, :, ch * 512:(ch + 1) * 512],
                                start=(p == 0), stop=(p == NP - 1),
                                perf_mode=DR)
                        nc.vector.reciprocal_approx_fast(
                            rinv[:, ch * 512:(ch + 1) * 512], ps_r[:])
                    # evacuate PV psums (unnormalized) to free the banks
                    ot_u = oupool.tile([128, N], bf16, tag="otu", name="ot_u")
                    for ch in range(NCH):
                        nc.vector.tensor_copy(
                            ot_u[:, ch * 512:(ch + 1) * 512], accs[ch][:])
                    rb = rpool.tile([128, N], f32, tag="rb", name="rb")
                    for ch in range(NCH):
                        nc.gpsimd.partition_broadcast(
                            rb[:, ch * 512:(ch + 1) * 512],
                            rinv[:, ch * 512:(ch + 1) * 512], channels=128)
                    nc.vector.tensor_mul(ot_t[img][:, h, :], ot_u[:], rb[:])

                return finish

            def alloc_img(img):
                qk_t[img] = qkpool.tile([128, 2 * CT, N], QKV_D, tag=f"qk{img}",
                                        name=f"qk{img}")
                vt_t[img] = vtpool.tile([128, NT, C], fp8, tag=f"vt{img}",
                                        name=f"vt{img}")
                ot_t[img] = otpool.tile([128, NH, N], fp8, tag=f"ot{img}",
                                        name=f"ot{img}")

            # ---------- projection + residual ----------
            def proj_mm_group(img, pt_i, t, ps):
                ot = ot_t[img]
                for ch in range(NCH):
                    nc.tensor.matmul(
                        ps[ch][:],
                        wp_sb[t][:, :, pt_i * 128:(pt_i + 1) * 128],
                        ot[:, 2 * t:2 * t + 2, ch * 512:(ch + 1) * 512],
                        start=(t == 0), stop=(t == KP - 1), perf_mode=DR)

            def proj_mms(img, pt_i, zip_pool=False):
                if zip_pool:
                    psa = accps.tile([128, 512], f32, tag="accr",
                                     name=f"pps{img}_{pt_i}a", bufs=2)
                    psb = accps.tile([128, 512], f32, tag="accr",
                                     name=f"pps{img}_{pt_i}b", bufs=2)
                    ps = [psa, psb]
                else:
                    pst = mmps.tile([128, N], f32, tag="mm",
                                    name=f"pps{img}_{pt_i}")
                    ps = [pst[:, 0:512], pst[:, 512:1024]]
                proj_mm_group(img, pt_i, 0, ps)
                return ps

            def proj_fin(img, pt_i, ps):
                proj_mm_group(img, pt_i, 1, ps)
                outt = outpool.tile([128, N], f32, tag="outt",
                                    name=f"o{img}_{pt_i}")
                for ch in range(NCH):
                    nc.vector.scalar_tensor_tensor(
                        out=outt[:, ch * 512:(ch + 1) * 512],
                        in0=ps[ch][:],
                        scalar=pb_sb[:, pt_i:pt_i + 1],
                        in1=xts[img][:, pt_i, ch * 512:(ch + 1) * 512],
                        op0=OP.add, op1=OP.add)
                for ch in range(NCH):
                    nc.sync.dma_start(
                        out_d[img, pt_i * 128:(pt_i + 1) * 128,
                              ch * 512:(ch + 1) * 512],
                        outt[:, ch * 512:(ch + 1) * 512])

            def proj_block(img, pt_i):
                proj_fin(img, pt_i, proj_mms(img, pt_i, zip_pool=(img == 0)))

            def gen_proj(img):
                for pt_i in range(CT):
                    proj_block(img, pt_i)
                    yield

            def gn0_batched():
                """GN for img0 with all 4 cts' group math vectorized: the
                ~350ns/op in-order DVE queue cost makes per-ct serial chains
                the startup pacer."""
                img = 0
                xn_t[img] = xnpool.tile([128, CT, N], QKV_D, tag=f"xn{img}",
                                        name=f"xn{img}")
                xt = xts[img]
                s2a = spool.tile([128, 2 * CT], f32, tag="s2a", bufs=1)
                for ct in range(CT):
                    st = spool.tile([128, 2, 6], f32, tag="bnst", name="st")
                    nc.vector.bn_stats(st[:, 0, :], xt[:, ct, 0:512])
                    nc.vector.bn_stats(st[:, 1, :], xt[:, ct, 512:1024])
                    mv = spool.tile([128, 2], f32, tag="mv", name="mv")
                    nc.vector.bn_aggr(mv[:], st[:])
                    nc.vector.tensor_copy(s2a[:, 2 * ct:2 * ct + 1],
                                          mv[:, 0:1])
                    nc.vector.tensor_mul(s2a[:, 2 * ct + 1:2 * ct + 2],
                                         mv[:, 0:1], mv[:, 0:1])
                    nc.vector.tensor_add(s2a[:, 2 * ct + 1:2 * ct + 2],
                                         s2a[:, 2 * ct + 1:2 * ct + 2],
                                         mv[:, 1:2])
                psg = accps.tile([2, 2 * CT], f32, tag="accr", name="psg0", bufs=2)
                nc.tensor.matmul(psg[:], sel[:], s2a[:], start=True, stop=True)
                gs = spool.tile([2, 2 * CT], f32, tag="gs0", bufs=1)
                nc.vector.tensor_copy(gs[:], psg[:])
                gs3 = gs[:].rearrange("p (t s) -> p t s", s=2)
                var_g = spool.tile([2, CT], f32, tag="gvar0", bufs=1)
                nc.vector.tensor_mul(var_g[:], gs3[:, :, 0], gs3[:, :, 0])
                nc.vector.tensor_sub(var_g[:], gs3[:, :, 1], var_g[:])
                nc.vector.tensor_scalar(
                    out=var_g[:], in0=var_g[:], scalar1=eps_t[:], scalar2=None,
                    op0=OP.add)
                r = spool.tile([2, CT], f32, tag="gnr0", bufs=1)
                nc.vector.tensor_scalar(
                    out=r[:], in0=var_g[:], scalar1=c_mh[:], scalar2=c_32[:],
                    op0=OP.mult, op1=OP.add)
                t1 = spool.tile([2, CT], f32, tag="gnt0", bufs=1)
                nc.vector.tensor_mul(t1[:], r[:], r[:])
                nc.vector.tensor_mul(t1[:], t1[:], var_g[:])
                nc.vector.tensor_scalar(
                    out=t1[:], in0=t1[:], scalar1=c_mh[:], scalar2=c_32[:],
                    op0=OP.mult, op1=OP.add)
                nc.vector.tensor_mul(r[:], r[:], t1[:])
                mr = spool.tile([2, 2 * CT], f32, tag="mr0", bufs=1)
                mr3 = mr[:].rearrange("p (t s) -> p t s", s=2)
                nc.vector.tensor_copy(mr3[:, :, 0], gs3[:, :, 0])
                nc.vector.tensor_copy(mr3[:, :, 1], r[:])
                mubc = accps.tile([128, 2 * CT], f32, tag="accr", name="mubc0", bufs=2)
                nc.tensor.matmul(mubc[:], selB[:], mr[:], start=True,
                                 stop=True)
                mu3 = mubc[:].rearrange("p (t s) -> p t s", s=2)
                a_a = spool.tile([128, CT], f32, tag="aa0", bufs=1)
                nc.vector.tensor_mul(a_a[:], mu3[:, :, 1], gam_sb[:])
                b_a = spool.tile([128, CT], f32, tag="ba0", bufs=1)
                nc.vector.tensor_mul(b_a[:], mu3[:, :, 0], a_a[:])
                nc.vector.tensor_sub(b_a[:], bet_sb[:], b_a[:])
                for ct in range(CT):
                    nc.vector.tensor_scalar(
                        out=xn_t[img][:, ct, 0:512], in0=xt[:, ct, 0:512],
                        scalar1=a_a[:, ct:ct + 1], scalar2=b_a[:, ct:ct + 1],
                        op0=OP.mult, op1=OP.add)
                    nc.scalar.activation(
                        xn_t[img][:, ct, 512:1024], xt[:, ct, 512:1024],
                        AF.Identity, scale=a_a[:, ct:ct + 1],
                        bias=b_a[:, ct:ct + 1])

            # ================= emission schedule =================
            alloc_img(0)
            alloc_img(1)
            # GN0 inline (critical path), batched
            gn0_batched()
            # head0 prerequisites: q,k slots 0 and 4 (v0 is zipped)
            qk_block(0, 0, on_act=True)
            qk_block(0, 4, on_act=True)
            # zip queue: img0 v + rest of img0 qk, then gn1 + img1 qkv.
            # Zipped qk evacuations ALTERNATE between ACT and DVE so neither
            # in-order queue stalls the exp stream or the S-matmul inputs.
            zipq.append(gen_v(0, range(NT)))
            zipq.append(gen_qk(0, [1, 5], on_act=True))
            zipq.append(gen_qk(0, [2, 6], on_act=False))
            zipq.append(gen_gn(1))
            zipq.append(gen_qk(0, [3, 7], on_act=True))
            zipq.append(gen_qk(1, [0, 4], on_act=False))
            zipq.append(gen_v(1, range(0, 6)))
            zipq.append(gen_qk(1, [1, 5], on_act=True))
            for h in range(NH):
                attn_head(0, h, zip_per_pair=3)()
            # attn1: next heads' qk first, then v tail, proj0, last head's qk
            zipq.append(gen_qk(1, [2, 6], on_act=False))
            zipq.append(gen_v(1, range(6, NT)))
            zipq.append(gen_proj(0))
            zipq.append(gen_qk(1, [3, 7], on_act=True))
            for h in range(NH):
                attn_head(1, h, zip_per_pair=2)()
            drain()
            # tail: interleave proj1 blocks pairwise -- the first
            # accumulation step (head-pair 0) doesn't depend on the last
            # head's normalize chain, so it runs while that chain drains
            for b0, b1 in [(0, 1), (2, 3)]:
                ps0 = proj_mms(1, b0)
                ps1 = proj_mms(1, b1)
                proj_fin(1, b0, ps0)
                proj_fin(1, b1, ps1)

    nc.compile()
    return nc


_NC_CACHE = None


def _get_nc():
    global _NC_CACHE
    if _NC_CACHE is None:
        _NC_CACHE = build_program()
    return _NC_CACHE


def _host_prep(x, norm_gamma, norm_beta, qkv_w, qkv_b, proj_w, proj_b):
    qkv_w = np.ascontiguousarray(qkv_w, dtype=np.float32)
    proj_w = np.ascontiguousarray(proj_w, dtype=np.float32)
    wqkT = qkv_w[:2 * C].T          # [c, o] = [512, 1024]
    wvT = qkv_w[2 * C:].T           # [512, 512]
    wpT = proj_w.T                  # [512, 512]
    if FP8_QKV:
        wqk = np.ascontiguousarray(
            wqkT.reshape(KP, 2, 128, 2 * C).transpose(0, 2, 1, 3)).astype(NP8)
        wv = np.ascontiguousarray(
            wvT.reshape(KP, 2, 128, C).transpose(0, 2, 1, 3)).astype(NP8)
    else:
        wqk = np.ascontiguousarray(wqkT.reshape(CT, 128, 2 * C)).astype(NPBF)
        wv = np.ascontiguousarray(wvT.reshape(CT, 128, C)).astype(NPBF)
    wp = np.ascontiguousarray(
        wpT.reshape(KP, 2, 128, C).transpose(0, 2, 1, 3)).astype(NP8)
    common = {
        "wqk": wqk, "wv": wv, "wp": wp,
        "qkb": np.ascontiguousarray(qkv_b[:2 * C], dtype=np.float32),
        "vb": np.ascontiguousarray(qkv_b[2 * C:], dtype=np.float32),
        "pb": np.ascontiguousarray(proj_b, dtype=np.float32),
        "gamma": np.ascontiguousarray(norm_gamma, dtype=np.float32),
        "beta": np.ascontiguousarray(norm_beta, dtype=np.float32),
    }
    xr = np.ascontiguousarray(np.asarray(x, dtype=np.float32).reshape(B, C, N))
    in_maps = []
    for c in range(N_CORES):
        m = dict(common)
        m["x"] = np.ascontiguousarray(xr[c * B_LOC:(c + 1) * B_LOC])
        in_maps.append(m)
    return in_maps


def run(inputs, trace=False):
    nc = _get_nc()
    in_maps = _host_prep(**inputs)
    res = None
    for attempt in range(3):
        try:
            res = run_bass_kernel_spmd(
                nc, in_maps, core_ids=list(range(N_CORES)), trace=trace)
            break
        except Exception:
            if attempt == 2:
                raise
    parts = [res.results[c]["out"] for c in range(N_CORES)]
    out = np.concatenate(parts, axis=0).reshape(B, C, HH, WW)
    return out.astype(np.float32), res


def kernel(**inputs):
    out, _ = run(inputs, trace=False)
    return out

